# revision 3
# baseline (speedup 1.0000x reference)
"""AffinityPropagate Trainium2 kernel, v2.

24 iterations of an 8-neighbor gated stencil:
    d <- (1-mask) * sum_k(gsh_k * shift_k(d)) / wsum + mask * blur

Strategy (8 NeuronCores, pure data parallel: one batch image per core):
  * Image [352, 1216] flattened row-major into SBUF [128 part x 3344].
  * Zero-padded 2D shifts == flat 1D shifted reads: per-direction gate
    weights are exactly 0 wherever a neighbor is out of bounds, so the
    row-wrap values the flat shift drags in are annihilated.
  * Per-direction weights W_k = |g_k shifted| * (1-mask)/wsum (fp16) are
    precomputed once into one [128, 8, F] tile.
  * Per iteration: DVE computes 7 of the 8 product planes (3 two-direction
    pair instructions via 3-dim APs + 1 single), GpSimd (Pool) computes the
    8th; PE sums the 8 product planes + b via identity-matmul PSUM
    accumulation; ACT casts PSUM->fp16 into ping-pong d tiles; the
    +-1218-element halos are refreshed with 2 SBUF->SBUF DMAs.
  * The TimelineSim cost model charges DVE 2x mode for any packed fp16
    operands regardless of byte alignment, so no aligned d_odd copy is kept.
"""

import numpy as np

import bass_rust
from concourse import bass, mybir
from concourse.bass_utils import run_bass_kernel_spmd
from concourse.tile import TileContext

B, H, W = 8, 352, 1216
HW = H * W            # 428032
P = 128
F = HW // P           # 3344
HALO = 1218           # > max |shift| (1217), even
DW = HALO + F + HALO  # d tile width
PROP_TIME = 24
NCHUNK = 4            # product chunks per iteration
CD = F // NCHUNK      # 836
CP = 418              # one PSUM bank
NSUB = CD // CP       # 2

# storage order of weight planes in wt3 (first 7 on DVE, last on Pool);
# DVE pairs: (0,1) d-delta 1, (2,3) d-delta 2, (4,5) d-delta 1; single: 6
WSHIFTS = [-1216, -1215, -1, 1, 1215, 1216, 1217, -1217]
# guidance channel k for each reference offset (dy*W+dx), reference order
REF_SHIFTS = [1217, 1216, 1215, 1, -1, -1215, -1216, -1217]
CHAN_FOR_SHIFT = {s: k for k, s in enumerate(REF_SHIFTS)}
DVE_PAIRS = [(0, 1), (2, 3), (4, 5)]
DVE_SINGLE = 6
POOL_DIR = 7

f32 = mybir.dt.float32
f16 = mybir.dt.float16
MULT = mybir.AluOpType.mult
ADD = mybir.AluOpType.add

_CACHE = {}


def _split_sync_waits(nc, max_waits=1):
    """The walrus in this container accepts at most one sync-wait command
    per instruction; hoist extras onto preceding same-engine no-ops."""
    for f in nc.m.functions:
        for bb in f.blocks:
            out = []
            for inst in bb.instructions:
                si = inst.sync_info
                if si is not None and si.on_wait and len(si.on_wait) > max_waits:
                    waits = list(si.on_wait)
                    carry, keep = waits[:-max_waits], waits[-max_waits:]
                    for j, w in enumerate(carry):
                        out.append(mybir.InstNoOp(
                            name=f"{inst.name}-ws{j}", engine=inst.engine,
                            sync_info=mybir.SyncInfo(on_wait=[w], on_update=[]),
                            bass_nofuse=True))
                    inst.sync_info = mybir.SyncInfo(
                        on_wait=keep, on_update=list(si.on_update))
                out.append(inst)
            bb.instructions[:] = out


def _pair_read_ap(dtile, start, delta, width):
    """AP reading dtile[:, start : start+width] and
    dtile[:, start+delta : start+delta+width] as a [P, 2, width] view."""
    s = dtile[:, start:start + width]
    pstride = s.ap[0][0]
    return bass_rust.AP(tensor=s.tensor, offset=s.offset,
                        ap=[[pstride, P], [delta, 2], [1, width]])


def _bcast_k_ap(tile2d, start, width, nk):
    """AP reading tile2d[:, start:start+width] broadcast over a middle
    k-dim of size nk -> [P, nk, width] view."""
    s = tile2d[:, start:start + width]
    pstride = s.ap[0][0]
    return bass_rust.AP(tensor=s.tensor, offset=s.offset,
                        ap=[[pstride, P], [0, nk], [1, width]])


def _emit_shifted_plane_load(nc, gst, g, k, s, zrow):
    """gst[p, j] <- g[k, p*F + j + s], with rows whose 2D source row is out
    of bounds forced to zero (wrap columns are handled via mask multiplies)."""
    engs = (nc.sync, nc.scalar)
    eng = engs[k % 2]
    if s >= 0:
        for i, (p0, p1) in enumerate(((0, 32), (32, 64), (64, 96), (96, 127))):
            engs[(k + i) % 2].dma_start(
                out=gst[p0:p1, :],
                in_=g[k, s + p0 * F:s + p1 * F].rearrange(
                    "(p f) -> p f", p=p1 - p0))
        if s > 0:
            eng.dma_start(
                out=gst[127:128, 0:F - s],
                in_=g[k, s + 127 * F:HW].rearrange("(p f) -> p f", p=1))
        else:
            eng.dma_start(
                out=gst[127:128, :],
                in_=g[k, 127 * F:HW].rearrange("(p f) -> p f", p=1))
    else:
        a = -s
        eng.dma_start(
            out=gst[0:1, a:F],
            in_=g[k, 0:F - a].rearrange("(p f) -> p f", p=1))
        for i, (p0, p1) in enumerate(((1, 32), (32, 64), (64, 96), (96, 128))):
            engs[(k + i) % 2].dma_start(
                out=gst[p0:p1, :],
                in_=g[k, p0 * F - a:p1 * F - a].rearrange(
                    "(p f) -> p f", p=p1 - p0))
    # top/bottom image rows (dy out of bounds) + DMA-uncovered slivers.
    if s in (-1217, -1216, -1215):          # dy = -1
        nc.vector.memset(gst[0:1, 0:max(1216, -s)], 0.0)
    elif s == -1:
        nc.vector.memset(gst[0:1, 0:1], 0.0)
    elif s in (1215, 1216, 1217):           # dy = +1
        start = min(F - 1216, F - s)
        eng.dma_start(out=gst[127:128, start:F], in_=zrow[0:1, 0:F - start])
    elif s == 1:
        eng.dma_start(out=gst[127:128, F - 1:F], in_=zrow[0:1, 0:1])


def _build():
    nc = bass.Bass()
    g = nc.dram_tensor("g", [8, HW], f32, kind="ExternalInput")
    blur = nc.dram_tensor("blur", [HW], f32, kind="ExternalInput")
    sparse = nc.dram_tensor("sparse", [HW], f32, kind="ExternalInput")
    maskL = nc.dram_tensor("maskL", [P, F], f16, kind="ExternalInput")
    maskR = nc.dram_tensor("maskR", [P, F], f16, kind="ExternalInput")
    ident = nc.dram_tensor("ident", [P, P], f16, kind="ExternalInput")
    out = nc.dram_tensor("out", [P, F], f32, kind="ExternalOutput")

    with TileContext(nc) as tc:
        with tc.tile_pool(name="const", bufs=1) as constp, \
             tc.tile_pool(name="wpool", bufs=1) as wpool, \
             tc.tile_pool(name="dpool", bufs=1) as dpool, \
             tc.tile_pool(name="misc", bufs=1) as miscp:

            identt = constp.tile([P, P], f16)
            nc.sync.dma_start(out=identt[:], in_=ident[:])
            zrow = constp.tile([P, 1220], f32)
            nc.gpsimd.memset(zrow[:], 0.0)

            wt3 = wpool.tile([P, 8, F], f16, name="wt3")
            bt = miscp.tile([P, F], f16)

            dA = dpool.tile([P, DW], f16, tag="dA")
            dB = dpool.tile([P, DW], f16, tag="dB")
            for t in (dA, dB):
                nc.gpsimd.memset(t[:, 0:HALO], 0.0)
                nc.gpsimd.memset(t[:, HALO + F:DW], 0.0)

            # ---------------- preprocessing ----------------
            with tc.tile_pool(name="pre", bufs=2) as prep, \
                 tc.tile_pool(name="psumpre", bufs=4, space="PSUM") as psumpre:
                maskLt = prep.tile([P, F], f16, tag="mL", bufs=1)
                maskRt = prep.tile([P, F], f16, tag="mR", bufs=1)
                nc.sync.dma_start(out=maskLt[:], in_=maskL[:])
                nc.scalar.dma_start(out=maskRt[:], in_=maskR[:])
                sparse_st = prep.tile([P, F], f32, tag="sp32", bufs=1)
                blur_st = prep.tile([P, F], f32, tag="bl32", bufs=1)
                sign16 = prep.tile([P, F], f16, tag="m", bufs=1)
                m16inv = prep.tile([P, F], f16, tag="minv", bufs=1)

                # wsum accumulated on PE from masked gate planes
                psw = [psumpre.tile([P, CP], f32, name=f"psw{q}", bufs=1,
                                    tag=f"psw{q}") for q in range(8)]
                # blur/sparse first: d0 and the sign -> (1-m) chain complete
                # early, far off the guidance-load critical path
                nc.sync.dma_start(
                    out=blur_st[:],
                    in_=blur[:].rearrange("(p f) -> p f", p=P))
                nc.scalar.dma_start(
                    out=sparse_st[:],
                    in_=sparse[:].rearrange("(p f) -> p f", p=P))
                # d0 = blur (fp16 body via ACT; halos via the SWDGE queue so
                # they cannot head-of-line block the guidance loads)
                nc.scalar.copy(out=dA[:, HALO:HALO + F], in_=blur_st[:])
                nc.gpsimd.dma_start(out=dA[1:128, 0:HALO],
                                    in_=dA[0:127, F:F + HALO])
                nc.gpsimd.dma_start(out=dA[0:127, HALO + F:DW],
                                    in_=dA[1:128, HALO:2 * HALO])
                nc.scalar.sign(sign16[:], sparse_st[:])
                nc.vector.tensor_scalar(m16inv[:], sign16[:], -1.0,
                                        1.0, MULT, ADD)
                nc.vector.tensor_tensor(bt[:], sign16[:], blur_st[:], MULT)

                # load order: Pool-masked dirs first (longest post-chain),
                # a no-mask dir (dx=0) last
                order = [7, 1, 2, 3, 4, 6, 0, 5]  # wt3 positions
                npool_masks = 0
                for i, pos in enumerate(order):
                    s = WSHIFTS[pos]
                    last_plane = i == 7
                    gst = prep.tile([P, F], f32, tag="gst", bufs=3)
                    if last_plane:
                        # final plane (shift +1216, no mask) loads in column
                        # chunks so recip/finalize chunk 0 starts without
                        # waiting for the whole plane
                        assert s == 1216
                        chan = CHAN_FOR_SHIFT[s]
                        for cc in range(NCHUNK):
                            a = s + cc * CD
                            u = g[chan, 0:1]
                            src = bass_rust.AP(
                                tensor=u.tensor, offset=u.offset + a,
                                ap=[[F, 127], [1, CD]])
                            engs = (nc.sync, nc.scalar)
                            engs[cc % 2].dma_start(out=gst[0:127, cc * CD:
                                                           (cc + 1) * CD],
                                                   in_=src)
                            # partition 127: valid source only to col F-s
                            lim = F - s  # 2128
                            lo, hi = cc * CD, (cc + 1) * CD
                            if lo < lim:
                                v = min(hi, lim)
                                u2 = g[chan, 0:1]
                                src2 = bass_rust.AP(
                                    tensor=u2.tensor,
                                    offset=u2.offset + s + 127 * F + lo,
                                    ap=[[F, 1], [1, v - lo]])
                                engs[cc % 2].dma_start(
                                    out=gst[127:128, lo:v], in_=src2)
                                if v < hi:
                                    engs[cc % 2].dma_start(
                                        out=gst[127:128, v:hi],
                                        in_=zrow[0:1, 0:hi - v])
                            else:
                                engs[cc % 2].dma_start(
                                    out=gst[127:128, lo:hi],
                                    in_=zrow[0:1, 0:CD])
                            nc.scalar.activation(
                                wt3[:, pos, lo:hi], gst[:, lo:hi],
                                mybir.ActivationFunctionType.Abs)
                            for q in range(2 * cc, 2 * cc + 2):
                                qs = q * CP
                                nc.tensor.matmul(psw[q][:], identt[:],
                                                 wt3[:, pos, qs:qs + CP],
                                                 start=False, stop=True)
                        continue
                    _emit_shifted_plane_load(nc, gst, g, CHAN_FOR_SHIFT[s], s,
                                             zrow)
                    # |g| -> fp16 gate plane in wt3
                    nc.scalar.activation(wt3[:, pos, :], gst[:],
                                         mybir.ActivationFunctionType.Abs)
                    dx = 1 if s in (-1215, 1, 1217) else \
                        (-1 if s in (-1217, -1, 1215) else 0)
                    if dx != 0:
                        mt = maskLt if dx == -1 else maskRt
                        # two of the six wrap masks run on the idle Pool
                        if npool_masks < 2:
                            npool_masks += 1
                            nc.gpsimd.tensor_tensor(
                                wt3[:, pos, :], wt3[:, pos, :], mt[:], MULT)
                        else:
                            nc.vector.tensor_tensor(
                                wt3[:, pos, :], wt3[:, pos, :], mt[:], MULT)
                    for q in range(8):
                        qs = q * CP
                        nc.tensor.matmul(psw[q][:], identt[:],
                                         wt3[:, pos, qs:qs + CP],
                                         start=(i == 0), stop=False)

                # winv' = (1-mask)/wsum, fp16; then W_k *= winv' in place
                winv = prep.tile([P, F], f16, tag="winv", bufs=1)
                for c in range(NCHUNK):
                    sl = slice(c * CD, (c + 1) * CD)
                    for q in range(c * CD // CP, (c + 1) * CD // CP):
                        qs = q * CP
                        with nc.allow_low_precision(
                                reason="fp16 weights absorb 1/wsum"):
                            nc.vector.reciprocal(winv[:, qs:qs + CP], psw[q][:])
                    nc.vector.tensor_tensor(winv[:, sl], winv[:, sl],
                                            m16inv[:, sl], MULT)
                    # W_k = gate_k * (1-mask)/wsum  (2 planes per instr;
                    # the Pool-owned plane 7 scales on Pool)
                    for p in range(3):
                        nc.vector.tensor_tensor(
                            wt3[:, 2 * p:2 * p + 2, sl],
                            wt3[:, 2 * p:2 * p + 2, sl],
                            _bcast_k_ap(winv, c * CD, CD, 2), MULT)
                    nc.vector.tensor_tensor(
                        wt3[:, 6, sl], wt3[:, 6, sl], winv[:, sl], MULT)
                    nc.gpsimd.tensor_tensor(
                        wt3[:, 7, sl], wt3[:, 7, sl], winv[:, sl], MULT)

            # ---------------- 24 stencil iterations ----------------
            with tc.tile_pool(name="prod", bufs=3) as prodp, \
                 tc.tile_pool(name="psum", bufs=6, space="PSUM") as psump, \
                 tc.tile_pool(name="post", bufs=1) as postp:

                src, dst = dA, dB
                ostage = postp.tile([P, F], f32)
                sp = WSHIFTS[POOL_DIR]
                ss = WSHIFTS[DVE_SINGLE]
                # per-iteration piece list (col_start, width): the final
                # chunk splits into two PSUM-bank halves so the next
                # iteration's first products wait on a shorter ACT tail
                even_pieces = [(0, CD), (CD, CD), (2 * CD, CD), (3 * CD, CD)]
                odd_pieces = [(3 * CD, CD), (2 * CD, CD), (CD, CD), (0, CD)]
                for it in range(PROP_TIME):
                    last = it == PROP_TIME - 1
                    pieces = even_pieces if it % 2 == 0 else odd_pieces
                    back_done = front_done = False
                    covered = []
                    for (cs, cw) in pieces:
                        pr = prodp.tile([P, 8, cw], f16,
                                        tag=f"pr{cw}", name=f"pr{cw}")
                        # Pool owns dir 7 everywhere and dir 6 on half of
                        # the columns; DVE covers dir 6's other half
                        nc.gpsimd.tensor_tensor(
                            pr[:, POOL_DIR, :], wt3[:, POOL_DIR, cs:cs + cw],
                            src[:, HALO + sp + cs:HALO + sp + cs + cw], MULT)
                        h6 = (cw * 29) // 41  # ~0.707: equalize DVE/Pool
                        nc.gpsimd.tensor_tensor(
                            pr[:, DVE_SINGLE, 0:h6],
                            wt3[:, DVE_SINGLE, cs:cs + h6],
                            src[:, HALO + ss + cs:HALO + ss + cs + h6], MULT)
                        nc.vector.tensor_tensor(
                            pr[:, DVE_SINGLE, h6:cw],
                            wt3[:, DVE_SINGLE, cs + h6:cs + cw],
                            src[:, HALO + ss + cs + h6:HALO + ss + cs + cw],
                            MULT)
                        pairs = DVE_PAIRS if it % 2 == 0 else DVE_PAIRS[::-1]
                        for (k1, k2) in pairs:
                            d1, d2 = WSHIFTS[k1], WSHIFTS[k2]
                            nc.vector.tensor_tensor(
                                pr[:, k1:k1 + 2, :],
                                wt3[:, k1:k1 + 2, cs:cs + cw],
                                _pair_read_ap(src, HALO + d1 + cs,
                                              d2 - d1, cw), MULT)
                        for h in range(cw // CP):
                            hs = h * CP
                            ps = psump.tile([P, CP], f32)
                            # seed the bank with b on ACT; matmuls accumulate
                            nc.scalar.copy(out=ps[:],
                                           in_=bt[:, cs + hs:cs + hs + CP])
                            for k in range(8):
                                nc.tensor.matmul(ps[:], identt[:],
                                                 pr[:, k, hs:hs + CP],
                                                 start=False, stop=(k == 7))
                            if last:
                                nc.scalar.copy(
                                    out=ostage[:, cs + hs:cs + hs + CP],
                                    in_=ps[:])
                                nc.sync.dma_start(
                                    out=out[:, cs + hs:cs + hs + CP],
                                    in_=ostage[:, cs + hs:cs + hs + CP])
                            else:
                                nc.scalar.copy(
                                    out=dst[:, HALO + cs + hs:HALO + cs + hs + CP],
                                    in_=ps[:])
                        if last:
                            continue
                        covered.append((cs, cs + cw))
                        def _covers(lo, hi):
                            pts = sorted(covered)
                            cur = lo
                            for a, b_ in pts:
                                if a <= cur < b_:
                                    cur = b_
                                if cur >= hi:
                                    return True
                            return cur >= hi
                        # halo refreshes as soon as their source cols land
                        if not back_done and _covers(0, HALO):
                            nc.sync.dma_start(
                                out=dst[0:127, HALO + F:DW],
                                in_=dst[1:128, HALO:2 * HALO])
                            back_done = True
                        if not front_done and _covers(F - HALO, F):
                            nc.scalar.dma_start(
                                out=dst[1:128, 0:HALO],
                                in_=dst[0:127, F:F + HALO])
                            front_done = True
                    src, dst = dst, src

    nc.finalize()
    _split_sync_waits(nc)
    return nc


def _consts():
    j = np.arange(HW, dtype=np.int64) % W
    mL = (j != 0).astype(np.float16).reshape(P, F)
    mR = (j != W - 1).astype(np.float16).reshape(P, F)
    return mL, mR, np.eye(P, dtype=np.float16)


def kernel(guidance, blur_depth, sparse_depth):
    if "nc" not in _CACHE:
        _CACHE["nc"] = _build()
    nc = _CACHE["nc"]
    guidance = np.asarray(guidance, dtype=np.float32)
    blur_depth = np.asarray(blur_depth, dtype=np.float32)
    sparse_depth = np.asarray(sparse_depth, dtype=np.float32)
    mL, mR, idm = _consts()
    in_maps = []
    for c in range(B):
        in_maps.append({
            "g": np.ascontiguousarray(guidance[c].reshape(8, HW)),
            "blur": np.ascontiguousarray(blur_depth[c].reshape(HW)),
            "sparse": np.ascontiguousarray(sparse_depth[c].reshape(HW)),
            "maskL": mL, "maskR": mR, "ident": idm,
        })
    # every iterate is a convex combination of blur_depth values, so the
    # output must stay inside blur's range; violations mean the device
    # glitched (transient NRT wedge) -> retry
    lo = float(blur_depth.min()) - 1e-2
    hi = float(blur_depth.max()) + 1e-2

    import time
    outp = None
    for attempt in range(4):
        try:
            res = run_bass_kernel_spmd(nc, in_maps, list(range(B)))
            outp = np.stack(
                [res.results[c]["out"].reshape(1, H, W) for c in range(B)])
            if np.isfinite(outp).all() and outp.min() >= lo and outp.max() <= hi:
                return outp
            print(f"kernel: attempt {attempt} produced out-of-range values; "
                  f"retrying", flush=True)
        except Exception as e:
            if attempt == 3:
                raise
            print(f"kernel: attempt {attempt} failed ({type(e).__name__}); "
                  f"retrying", flush=True)
        time.sleep(20 * (attempt + 1))
    return outp
